# revision 24
# baseline (speedup 1.0000x reference)
"""Trainium2 Bass kernel for BoundaryLoss.

loss = mean over pixels of BCE(pred_b, tgt_b) where pred_b/tgt_b are 0/1
Sobel-boundary maps of sigmoid(logits) / targets. Since both maps are
binary, the clamped BCE reduces exactly to 100 * mean(pred_b XOR tgt_b).

Strategy (pure data parallel over batch, 2 samples -> 8 images per core):
  - ALL images resident in SBUF (no buffer rotation, so no WAW stalls on
    the DMA queues; the queues stream back-to-back at full rate)
  - 4 DMAs per image spread over 3 queues: logits main chunks on the SP
    HWDGE ring, logits top chunk on the ACT HWDGE ring, targets on the
    gpsimd SWDGE ring with f32->f16 cast in the DMA datapath (frees DVE)
  - sigmoid (fp32 -> fp16) on ScalarE, writes the pred half of the
    interleaved pred/tgt tile
  - full 2D Sobel conv on TensorE in fp16: vertical band-matrix matmuls
    with column-shifted moving operands accumulating in PSUM. pred and
    tgt are interleaved in SBUF so ONE 1024-wide matmul computes both
    (5 matmuls per 126-row block instead of 10)
  - one 2048-wide square evacuation per block (PSUM -> f16), alternating
    ScalarE Square / VectorE self-multiply to balance engine load
  - margin m = (sqx - 0.25) + sqy via VectorE scalar_tensor_tensor;
    xor = (m_pred * m_tgt < 0); per-partition counts via tensor_scalar
    accum_out; final sum on host.
Each 512-row image = 4 main blocks of 126 output rows (input 128 rows
incl. 1-row halo) + an 8-row leftover; leftovers of all 8 images are
batched into one block-diagonal matmul set.
"""
import os
import numpy as np
import ml_dtypes

import concourse.bass as bass
import concourse.tile as tile
from concourse import bacc, mybir
from concourse.bass_utils import run_bass_kernel_spmd

F32 = mybir.dt.float32
F16 = mybir.dt.float16
BF16 = mybir.dt.bfloat16
AF = mybir.ActivationFunctionType
OP = mybir.AluOpType

B, C, H, W = 16, 4, 512, 512
N_CORES = 8
BPC = B // N_CORES          # batch entries per core
N_IMG = BPC * C             # images per core
MAIN_BLOCKS = [(0, 0, 127, 126), (126, 125, 128, 126),
               (252, 251, 128, 126), (378, 377, 128, 126)]
LEFT_IN, LEFT_OUT, LEFT_K, LEFT_M = 503, 504, 9, 8
N_SETS = N_IMG * len(MAIN_BLOCKS) + 1   # 33 count columns
CW = 514                                 # block chunk width incl pad cols
# fraction pattern of sets whose square-evacuation runs on ScalarE
_SQ_ACT_MOD = int(os.environ.get("BASS_SQ_ACT_MOD", "1"))
_SQ_ACT_LIM = int(os.environ.get("BASS_SQ_ACT_LIM", "1"))


# ---------------------------------------------------------------- bands
def _band_pair(in_rows, out_rows):
    K, M = len(in_rows), len(out_rows)
    vs = np.zeros((K, M), np.float32)
    vd = np.zeros((K, M), np.float32)
    for k, ir in enumerate(in_rows):
        for m, orow in enumerate(out_rows):
            d = ir - orow
            if d == 0:
                vs[k, m] = 2.0
            elif abs(d) == 1:
                vs[k, m] = 1.0
                vd[k, m] = float(d)
    return vs, vd


def _build_band_tensor():
    """Stack all band matrices into one [128, total_cols] array.
    offsets[(key, wname)] = (col, K, M)."""
    specs = {}
    specs['b0'] = _band_pair(range(0, 127), range(0, 126))
    specs['int'] = _band_pair(range(125, 253), range(126, 252))
    K, M = LEFT_K * N_IMG, LEFT_M * N_IMG
    vs = np.zeros((K, M), np.float32)
    vd = np.zeros((K, M), np.float32)
    svs, svd = _band_pair(range(LEFT_IN, 512), range(LEFT_OUT, 512))
    for i in range(N_IMG):
        vs[i*LEFT_K:(i+1)*LEFT_K, i*LEFT_M:(i+1)*LEFT_M] = svs
        vd[i*LEFT_K:(i+1)*LEFT_K, i*LEFT_M:(i+1)*LEFT_M] = svd
    specs['left'] = (vs, vd)

    cols = []
    offsets = {}
    col = 0
    for key, (vs, vd) in specs.items():
        for wname, wmat in (("vs", vs), ("vsn", -vs), ("vd2", 2.0*vd), ("vd", vd)):
            K, M = wmat.shape
            buf = np.zeros((128, M), np.float32)
            buf[:K, :] = wmat
            cols.append(buf)
            offsets[(key, wname)] = (col, K, M)
            col += M
    return np.concatenate(cols, axis=1), offsets


_BANDS, _BOFF = _build_band_tensor()
BANDW = _BANDS.shape[1]


# ---------------------------------------------------------------- kernel
_PIECES = os.environ.get("BASS_KERNEL_PIECES", "full")
_PS2048 = os.environ.get("BASS_PS2048", "0") == "1"
_COUNT_GP = os.environ.get("BASS_COUNT_GP", "0") == "1"


def _emit_set(nc, wsb, counts_sb, set_idx, rhs, K, M, band_key,
              psum_pool, sq_pool, m_pool, prod_pool, bias0):
    """Matmuls + squares + margins + xor count for one block-set.
    rhs: f16 AP [K, 2, CW] (pred at index 0, tgt at index 1 of dim 1);
    data at cols [1, 513), zero pad cols at 0 and 513."""
    if _PIECES in ("io", "dma"):
        return

    def wap(wname):
        col, kk, mm = _BOFF[(band_key, wname)]
        assert kk == K and mm == M
        return wsb[0:K, col:col + M]

    # shared sq tile per set, pair layout: [gx_p | gx_t | gy_p | gy_t];
    # per-tensor psum tiles (bufs=4) so release granularity stays fine
    sq = None if _PIECES == "conv" else sq_pool.tile([128, 2048], F16,
                                                     tag="sq")
    sq4 = None if sq is None else sq.rearrange("p (h t w) -> p h t w",
                                               h=2, t=2)
    if _PS2048:
        ps_big = psum_pool.tile([128, 2048], F32, tag="ps", name="ps_big")
    else:
        ps_big = None
    for t in (0, 1):
        src = rhs[:, t, :]
        if _PS2048:
            # pair layout in psum: [gx_p | gx_t | gy_p | gy_t]
            gx = ps_big[0:M, t*512:(t+1)*512]
            gy = ps_big[0:M, 1024+t*512:1024+(t+1)*512]
        else:
            ps = psum_pool.tile([128, 1024], F32, tag="ps")
            gx = ps[0:M, 0:512]
            gy = ps[0:M, 512:1024]
        # gx = Vs @ x[w+1] - Vs @ x[w-1]
        nc.tensor.matmul(gx, wap("vs"), src[:, 2:514], start=True, stop=False)
        nc.tensor.matmul(gx, wap("vsn"), src[:, 0:512], start=False, stop=True)
        # gy = Vd @ (x[w-1] + 2 x[w] + x[w+1])
        nc.tensor.matmul(gy, wap("vd2"), src[:, 1:513], start=True, stop=False)
        nc.tensor.matmul(gy, wap("vd"), src[:, 0:512], start=False, stop=False)
        nc.tensor.matmul(gy, wap("vd"), src[:, 2:514], start=False, stop=True)
        if _PIECES == "conv":
            continue
        # evacuate with square into pair-layout slots (strided out).
        # pred always on ACT; tgt on ACT or the DVE chain by pattern
        if _PS2048:
            continue
        sqt = sq4[0:M, :, t, :]
        if t == 0 or (set_idx % _SQ_ACT_MOD) < _SQ_ACT_LIM:
            nc.scalar.activation(sqt, ps[0:M, :], AF.Square,
                                 bias=bias0[0:M, 0:1])
        elif os.environ.get("BASS_DVE_POW", "0") == "1":
            nc.vector.tensor_scalar(sqt, ps[0:M, :], 2.0, None, OP.pow)
        else:
            cp = sq_pool.tile([128, 1024], F16, tag="cp")
            nc.vector.tensor_copy(cp[0:M, :], ps[0:M, :])
            nc.vector.tensor_tensor(sqt, cp[0:M, :], cp[0:M, :], OP.mult)
    if _PIECES == "conv":
        return
    if _PS2048:
        # single 2048-wide evac; psum already in pair layout matching sq
        nc.scalar.activation(sq[0:M, :], ps_big[0:M, :], AF.Square,
                             bias=bias0[0:M, 0:1])
    if _PIECES == "sq":
        return
    m = m_pool.tile([128, 1024], F16, tag="m")
    # margins for both tensors in ONE op: m = (gx^2 - 0.25) + gy^2
    nc.vector.scalar_tensor_tensor(m[0:M, :], sq[0:M, 0:1024], -0.25,
                                   sq[0:M, 1024:2048], OP.add, OP.add)
    prod = prod_pool.tile([128, 512], F16, tag="prod")
    nc.vector.tensor_tensor(prod[0:M, :], m[0:M, 0:512], m[0:M, 512:1024],
                            OP.mult)
    ind = prod_pool.tile([128, 512], F16, tag="ind")
    # out = (prod < 0); accum_out = per-partition sum(out) (op1 = reduce op)
    ceng = nc.gpsimd if _COUNT_GP else nc.vector
    ceng.tensor_scalar(ind[0:M, :], prod[0:M, :], 0.0, None, OP.is_lt,
                       OP.add,
                       accum_out=counts_sb[0:M, set_idx:set_idx+1])


def _build_nc(repeat: int = 1, loop_reps: int = 0):
    nc = bacc.Bacc("TRN2", target_bir_lowering=False, debug=False,
                   num_devices=N_CORES,
                   num_swdge_queues=int(os.environ.get("BASS_SWQ", "4")))
    logits = nc.declare_dram_parameter("logits", [BPC, C, H, W], F32,
                                       isOutput=False)
    targets = nc.declare_dram_parameter("targets", [BPC, C, H, W], F32,
                                        isOutput=False)
    bands = nc.declare_dram_parameter("bands", [128, BANDW], F16,
                                      isOutput=False)
    counts = nc.declare_dram_parameter("counts", [128, N_SETS], F32,
                                       isOutput=True)

    with tile.TileContext(nc) as tc:
        from contextlib import ExitStack
        with ExitStack() as ctx:
            consts = ctx.enter_context(tc.tile_pool(name="consts", bufs=1))
            psum_pool = ctx.enter_context(
                tc.tile_pool(name="psum", bufs=2 if _PS2048 else 4,
                             space="PSUM"))
            sq_pool = ctx.enter_context(tc.tile_pool(name="sqp", bufs=4))
            m_pool = ctx.enter_context(tc.tile_pool(name="mp", bufs=3))
            prod_pool = ctx.enter_context(tc.tile_pool(name="prodp", bufs=4))

            wsb = consts.tile([128, BANDW], F16)
            nc.sync.dma_start(out=wsb, in_=bands[:, :])
            bias0 = consts.tile([128, 1], F32)
            nc.vector.memset(bias0, 0.0)
            counts_sb = consts.tile([128, N_SETS], F32)
            nc.vector.memset(counts_sb, 0.0)

            # per-image resident tiles (fine-grained DMA/compute deps)
            lts, xts = [], []
            for i in range(N_IMG):
                lt_i = consts.tile([128, 4*CW], F32, name=f"lt{i}")
                xt_i = consts.tile([128, 4*2*CW], F16, name=f"xt{i}")
                x4 = xt_i.rearrange("p (c t w) -> p c t w", c=4, t=2)
                # zero pad columns (col 0 and 513 of each chunk, both halves)
                nc.vector.memset(x4[:, :, :, 0:1], 0.0)
                nc.vector.memset(x4[:, :, :, 513:514], 0.0)
                lts.append(lt_i)
                xts.append(xt_i)
            # leftover combined tiles
            lt_l = consts.tile([128, CW], F32, name="lt_l")
            xt_l = consts.tile([128, 2*CW], F16, name="xt_l")
            xt_l3 = xt_l.rearrange("p (t w) -> p t w", t=2)
            nc.vector.memset(xt_l3[:, :, 0:1], 0.0)
            nc.vector.memset(xt_l3[:, :, 513:514], 0.0)

            from contextlib import nullcontext
            loop_cm = (tc.For_i(0, loop_reps, 1) if loop_reps
                       else nullcontext())
            with loop_cm:
              for rep in range(repeat):
                set_idx = 0
                def emit_dmas(img):
                    b, c = divmod(img, C)
                    img_l = logits[b, c]
                    img_t = targets[b, c]
                    lt4 = lts[img].rearrange("p (c w) -> p c w", c=4)
                    xt4 = xts[img].rearrange("p (c t w) -> p c t w",
                                             c=4, t=2)
                    # all bulk loads on the SWDGE ring (HWDGE rings are
                    # ~4x slower for these 2KB-descriptor patterns);
                    # targets cast f32->f16 in the DMA datapath
                    nc.gpsimd.dma_start(out=lt4[0:128, 0, 1:513],
                                        in_=img_l[0:128, :])
                    src3l = bass.AP(
                        tensor=img_l.tensor,
                        offset=img_l.offset + 125 * W,
                        ap=[[W, 128], [126 * W, 3], [1, W]])
                    nc.gpsimd.dma_start(out=lt4[:, 1:4, 1:513], in_=src3l)
                    nc.gpsimd.dma_start(out=xt4[0:128, 0, 1, 1:513],
                                        in_=img_t[0:128, :])
                    src3t = bass.AP(
                        tensor=img_t.tensor,
                        offset=img_t.offset + 125 * W,
                        ap=[[W, 128], [126 * W, 3], [1, W]])
                    nc.gpsimd.dma_start(out=xt4[:, 1:4, 1, 1:513],
                                        in_=src3t)

                def emit_sigmoid(img):
                    if _PIECES == "dma":
                        return
                    lt4 = lts[img].rearrange("p (c w) -> p c w", c=4)
                    xt4 = xts[img].rearrange("p (c t w) -> p c t w",
                                             c=4, t=2)
                    nc.scalar.activation(xt4[:, :, 0, 1:513],
                                         lt4[:, :, 1:513],
                                         AF.Sigmoid, bias=bias0[:, 0:1])

                def emit_left_dmas():
                    src_left_l = bass.AP(
                        tensor=logits[0, 0].tensor,
                        offset=logits[0, 0].offset + LEFT_IN * W,
                        ap=[[H * W, N_IMG], [W, LEFT_K], [1, W]])
                    nc.gpsimd.dma_start(out=lt_l[0:N_IMG*LEFT_K, 1:513],
                                        in_=src_left_l)
                    src_left_t = bass.AP(
                        tensor=targets[0, 0].tensor,
                        offset=targets[0, 0].offset + LEFT_IN * W,
                        ap=[[H * W, N_IMG], [W, LEFT_K], [1, W]])
                    nc.gpsimd.dma_start(out=xt_l3[0:N_IMG*LEFT_K, 1, 1:513],
                                        in_=src_left_t)

                def emit_left_sigmoid():
                    if _PIECES == "dma":
                        return
                    nc.scalar.activation(xt_l3[0:N_IMG*LEFT_K, 0, 1:513],
                                         lt_l[0:N_IMG*LEFT_K, 1:513],
                                         AF.Sigmoid,
                                         bias=bias0[0:N_IMG*LEFT_K, 0:1])

                # software-pipelined emission: image 0's dma+sigmoid up
                # front; image k+1's sigmoid is emitted right after the
                # first set of image k so the in-order ACT queue never
                # makes PE wait a whole image of evacuations for it
                emit_dmas(0)
                emit_sigmoid(0)
                for img in range(N_IMG):
                    if img + 1 < N_IMG:
                        emit_dmas(img + 1)
                    xt4 = xts[img].rearrange("p (c t w) -> p c t w",
                                             c=4, t=2)
                    for blk, (ostart, istart, K, M) in enumerate(MAIN_BLOCKS):
                        _emit_set(nc, wsb, counts_sb, set_idx,
                                  xt4[0:K, blk, :, :], K, M,
                                  'b0' if ostart == 0 else 'int',
                                  psum_pool, sq_pool, m_pool, prod_pool,
                                  bias0)
                        set_idx += 1
                        if blk == 0 and img + 1 < N_IMG:
                            emit_sigmoid(img + 1)

                # leftover rows of all images, block-diagonal combined set
                emit_left_dmas()
                emit_left_sigmoid()
                KL, ML = LEFT_K * N_IMG, LEFT_M * N_IMG
                _emit_set(nc, wsb, counts_sb, set_idx, xt_l3[0:KL, :, :],
                          KL, ML, 'left', psum_pool, sq_pool, m_pool,
                          prod_pool, bias0)

            nc.sync.dma_start(out=counts[:, :], in_=counts_sb)
    nc.compile()
    return nc


_NC = None
LAST_RESULT = None


def kernel(logits: np.ndarray, targets: np.ndarray) -> np.ndarray:
    global _NC, LAST_RESULT
    if _NC is None:
        _NC = _build_nc()

    logits = np.ascontiguousarray(logits, dtype=np.float32)
    targets = np.ascontiguousarray(targets, dtype=np.float32)
    in_maps = []
    for c in range(N_CORES):
        in_maps.append({
            "logits": logits[c*BPC:(c+1)*BPC],
            "targets": targets[c*BPC:(c+1)*BPC],
            "bands": _BANDS.astype(np.float16),
        })
    res = run_bass_kernel_spmd(
        _NC, in_maps, list(range(N_CORES)),
        trace=bool(os.environ.get("BASS_TRACE_KERNEL")),
    )
    LAST_RESULT = res
    total_xor = 0.0
    for r in res.results:
        total_xor += float(np.asarray(r["counts"], dtype=np.float64).sum())
    loss = 100.0 * total_xor / float(B * C * H * W)
    return np.float32(loss)
